# revision 1
# baseline (speedup 1.0000x reference)
"""Trainium2 Bass kernel for an 8-head cross-attention block.

Math (per reference):
    Q = video @ Wq[h]           [4096, 64]  per head
    K = text  @ Wk[h]           [1024, 64]
    V = text  @ Wv[h]           [1024, 64]
    att = softmax(Q @ K^T)      [4096, 1024]   (no scaling)
    y_h = att @ V               [4096, 64]
    out = concat_h(y_h) @ Wout + pos_enc(4096, 512)

Sharding: head-parallel over 8 NeuronCores. Core h owns head h and the
matching 64 rows of Wout (row-parallel), producing a full [4096, 512]
partial output; the all-reduce over cores and the positional-encoding add
happen on host during the gather.

On-device layout: activations are kept "transposed" ([feature, token]) so
every TensorE contraction runs over the partition axis with no on-device
transposes. Softmax runs as exp(E - 12) (logits are O(12); the shift keeps
fp16 in range and cancels in the ratio) and the denominator comes for free
as a 65th output row of the att@V matmul via a ones-column appended to V.

Everything runs in fp16 operands (10-bit mantissa, full PE rate, half the
HBM traffic) with fp32 PSUM accumulation and an fp32 softmax denominator.
The two K=64 contraction stages (E^T and the output projection) keep their
operands duplicated on both PE partition halves so two matmuls occupy the
128-row array concurrently (row tiling via base_partition).
"""

import numpy as np

from concourse import bacc
import concourse.mybir as mybir
from concourse.tile import TileContext
from concourse.bass_utils import run_bass_kernel_spmd

N, M, D, H, DH = 4096, 1024, 512, 8, 64
P = 128
NC = 512          # n-chunk width for the attention pipeline
NJ = N // NC      # 8 n-chunks
DC = D // P       # 4 contraction chunks of 128
MT = M // P       # 8 key tiles of 128
F32 = mybir.dt.float32
FP16 = mybir.dt.float16
EXP = mybir.ActivationFunctionType.Exp
EXP_SHIFT = -12.0  # exp(E + shift): keeps exp in fp16 range; cancels in softmax
NCORES = 8

_CACHE: dict = {}
TRACE = False          # test harness can flip this before calling kernel()
LAST_RESULT = None     # BassKernelResults of the last run (for profiling)
DEBUG = False          # add intermediate dumps (dev only)


def _body(tc, nc, vT, tT, wq, wk, wv, wo, out, dscr):
    with tc.tile_pool(name="const", bufs=1) as cp:
        vt_sb = cp.tile([P, DC * N], FP16, tag="vt")
        tt_sb = cp.tile([P, DC * M], FP16, tag="tt")
        wq_sb = cp.tile([P, DC * DH], FP16, tag="wq")
        wk_sb = cp.tile([P, DC * DH], FP16, tag="wk")
        wv_sb = cp.tile([P, DC * DH], FP16, tag="wv")
        wo_sb = cp.tile([P, D], FP16, tag="wo")      # wo duplicated on both halves
        qt_sb = cp.tile([P, N], FP16, tag="qt")      # Q^T duplicated on both halves
        kt_sb = cp.tile([P, M], FP16, tag="kt")      # K^T duplicated on both halves
        v_sb = cp.tile([P, MT * (DH + 1)], FP16, tag="vsb")
        y_sb = cp.tile([P, N], FP16, tag="ysb")      # Y^T duplicated on both halves
        den_sb = cp.tile([1, N], F32, tag="den")
        rsrc = cp.tile([P, N // P], F32, tag="rsrc")
        rc_sb = cp.tile([P, N // P], F32, tag="rc")

        for c in range(DC):
            nc.sync.dma_start(out=tt_sb[:, c * M:(c + 1) * M], in_=tT[c * P:(c + 1) * P, :])
            nc.sync.dma_start(out=wq_sb[:, c * DH:(c + 1) * DH], in_=wq[c * P:(c + 1) * P, :])
            nc.sync.dma_start(out=wk_sb[:, c * DH:(c + 1) * DH], in_=wk[c * P:(c + 1) * P, :])
            nc.sync.dma_start(out=wv_sb[:, c * DH:(c + 1) * DH], in_=wv[c * P:(c + 1) * P, :])
        nc.sync.dma_start(out=wo_sb[0:DH, :], in_=wo[:, :])
        nc.sync.dma_start(out=wo_sb[DH:P, :], in_=wo[:, :])
        for c in range(DC):
            nc.sync.dma_start(out=vt_sb[:, c * N:(c + 1) * N], in_=vT[c * P:(c + 1) * P, :])

        v3 = v_sb.rearrange("p (m e) -> p m e", e=DH + 1)  # [128, 8, 65]
        nc.vector.memset(v3[:, :, DH], 1.0)
        bias_sb = cp.tile([P, 1], F32, tag="bias")
        nc.vector.memset(bias_sb[:, :], EXP_SHIFT)

        # ---- projections: K^T [64,1024], V' [128, 8x65], Q^T [64,4096] ----
        with tc.tile_pool(name="ps_proj", bufs=2, space="PSUM") as pj:
            for half in range(M // 512):
                ps = pj.tile([DH, 512], F32, tag="ps")
                for c in range(DC):
                    nc.tensor.matmul(
                        ps[:, :],
                        wk_sb[:, c * DH:(c + 1) * DH],
                        tt_sb[:, c * M + half * 512: c * M + (half + 1) * 512],
                        start=(c == 0), stop=(c == DC - 1))
                sl = slice(half * 512, (half + 1) * 512)
                nc.vector.tensor_copy(out=kt_sb[0:DH, sl], in_=ps[:, :])
                nc.vector.tensor_copy(out=kt_sb[DH:P, sl], in_=ps[:, :])
            for mt in range(MT):
                ps = pj.tile([P, DH], F32, tag="psv")
                for c in range(DC):
                    nc.tensor.matmul(
                        ps[:, :],
                        tt_sb[:, c * M + mt * P: c * M + (mt + 1) * P],
                        wv_sb[:, c * DH:(c + 1) * DH],
                        start=(c == 0), stop=(c == DC - 1))
                nc.vector.tensor_copy(out=v3[:, mt, 0:DH], in_=ps[:, :])
            for j in range(NJ):
                ps = pj.tile([DH, 512], F32, tag="ps")
                for c in range(DC):
                    nc.tensor.matmul(
                        ps[:, :],
                        wq_sb[:, c * DH:(c + 1) * DH],
                        vt_sb[:, c * N + j * NC: c * N + (j + 1) * NC],
                        start=(c == 0), stop=(c == DC - 1))
                sl = slice(j * NC, (j + 1) * NC)
                nc.vector.tensor_copy(out=qt_sb[0:DH, sl], in_=ps[:, :])
                nc.vector.tensor_copy(out=qt_sb[DH:P, sl], in_=ps[:, :])

        # ---- attention: E^T = K^T.T @ Q^T -> exp -> Y'^T = V'.T @ P^T ----
        # E matmuls are K=64: pack two per PE pass on partition halves.
        with tc.tile_pool(name="ps_e", bufs=3, space="PSUM") as pe_pool, \
             tc.tile_pool(name="ps_y", bufs=2, space="PSUM") as py_pool, \
             tc.tile_pool(name="p_sb", bufs=8) as p_pool:

            def emit_y(j, p_tiles):
                ps = py_pool.tile([DH + 1, NC], F32, tag="y")
                for mt in range(MT):
                    nc.tensor.matmul(
                        ps[:, :],
                        v3[:, mt, :],
                        p_tiles[mt // 2][:, (mt % 2) * 512:(mt % 2 + 1) * 512],
                        start=(mt == 0), stop=(mt == MT - 1))
                sl = slice(j * NC, (j + 1) * NC)
                nc.vector.tensor_copy(out=y_sb[0:DH, sl], in_=ps[0:DH, :])
                nc.vector.tensor_copy(out=y_sb[DH:P, sl], in_=ps[0:DH, :])
                # fp32 denominator row kept at full precision
                nc.vector.tensor_copy(out=den_sb[:, sl], in_=ps[DH:DH + 1, :])

            prev = None
            for j in range(NJ):
                jsl = slice(j * NC, (j + 1) * NC)
                p_tiles = []
                for pair in range(MT // 2):
                    mt = pair * 2
                    e_ps = pe_pool.tile([P, 1024], F32, tag="e")
                    nc.tensor.matmul(
                        e_ps[:, 0:512],
                        kt_sb[0:DH, mt * P:(mt + 1) * P],
                        qt_sb[0:DH, jsl],
                        start=True, stop=True)
                    nc.tensor.matmul(
                        e_ps[:, 512:1024],
                        kt_sb[DH:P, (mt + 1) * P:(mt + 2) * P],
                        qt_sb[DH:P, jsl],
                        start=True, stop=True)
                    pt = p_pool.tile([P, 1024], FP16, tag="p")
                    nc.scalar.activation(pt[:, :], e_ps[:, :], EXP, bias=bias_sb[:, :])
                    p_tiles.append(pt)
                if prev is not None:
                    emit_y(j - 1, prev)
                prev = p_tiles
            emit_y(NJ - 1, prev)

        # ---- denominator: [1, 4096] -> DRAM -> [128, 32] scatter -> recip ----
        # (a direct SBUF->SBUF partition-scatter DMA returns garbage on HW)
        nc.sync.dma_start(out=dscr[:], in_=den_sb[:, :])
        nc.sync.dma_start(out=rsrc[:, :], in_=dscr.rearrange("(t p) -> p t", p=P))
        nc.vector.reciprocal(rc_sb[:, :], rsrc[:, :])
        if DEBUG:
            dbg_den = nc.dram_tensor("dbg_den", [1, N], F32, kind="ExternalOutput")
            dbg_rsrc = nc.dram_tensor("dbg_rsrc", [P, N // P], F32, kind="ExternalOutput")
            dbg_rc = nc.dram_tensor("dbg_rc", [P, N // P], F32, kind="ExternalOutput")
            dbg_y = nc.dram_tensor("dbg_y", [P, N], FP16, kind="ExternalOutput")
            nc.sync.dma_start(out=dbg_den[:, :], in_=den_sb[:, :])
            nc.sync.dma_start(out=dbg_rsrc[:, :], in_=rsrc[:, :])
            nc.sync.dma_start(out=dbg_rc[:, :], in_=rc_sb[:, :])
            nc.sync.dma_start(out=dbg_y[:, :], in_=y_sb[:, :])

        # ---- output projection (K=64, packed two per PE pass) + scaling ----
        out_r = out.rearrange("(g p) d -> p g d", p=P)  # [128, 32, 512]
        with tc.tile_pool(name="ps_o", bufs=4, space="PSUM") as po_pool, \
             tc.tile_pool(name="o_sb", bufs=2) as o_pool:
            for g in range(N // P // 4):
                ot = o_pool.tile([P, 4 * D], FP16, tag="o")
                for k in range(0, 4, 2):
                    nt = g * 4 + k
                    ps_a = po_pool.tile([P, D], F32, tag="po")
                    ps_b = po_pool.tile([P, D], F32, tag="po")
                    nc.tensor.matmul(
                        ps_a[:, :],
                        y_sb[0:DH, nt * P:(nt + 1) * P],
                        wo_sb[0:DH, :],
                        start=True, stop=True)
                    nc.tensor.matmul(
                        ps_b[:, :],
                        y_sb[DH:P, (nt + 1) * P:(nt + 2) * P],
                        wo_sb[DH:P, :],
                        start=True, stop=True)
                    nc.vector.tensor_scalar_mul(
                        ot[:, k * D:(k + 1) * D], ps_a[:, :], rc_sb[:, nt:nt + 1])
                    nc.vector.tensor_scalar_mul(
                        ot[:, (k + 1) * D:(k + 2) * D], ps_b[:, :], rc_sb[:, nt + 1:nt + 2])
                nc.sync.dma_start(
                    out=out_r[:, g * 4:(g + 1) * 4, :],
                    in_=ot.rearrange("p (g d) -> p g d", d=D))


def _build():
    nc = bacc.Bacc("TRN2", target_bir_lowering=False, debug=False)
    vT = nc.dram_tensor("vT", [D, N], FP16, kind="ExternalInput")
    tT = nc.dram_tensor("tT", [D, M], FP16, kind="ExternalInput")
    wq = nc.dram_tensor("wq", [D, DH], FP16, kind="ExternalInput")
    wk = nc.dram_tensor("wk", [D, DH], FP16, kind="ExternalInput")
    wv = nc.dram_tensor("wv", [D, DH], FP16, kind="ExternalInput")
    wo = nc.dram_tensor("wo", [DH, D], FP16, kind="ExternalInput")
    out = nc.dram_tensor("out", [N, D], FP16, kind="ExternalOutput")
    dscr = nc.dram_tensor("dscr", [N], F32)
    with TileContext(nc) as tc:
        _body(tc, nc, vT[:, :], tT[:, :], wq[:, :], wk[:, :], wv[:, :],
              wo[:, :], out[:, :], dscr[:])
    nc.compile()
    return nc


def _pos_encoding():
    # Mirror the reference's jnp ops bit-for-bit (numpy's f32 sin/exp differ
    # by enough ULPs to dominate the error budget at pos/freq ~ 4e3).
    import jax
    import jax.numpy as jnp
    with jax.default_device(jax.devices("cpu")[0]):
        pos = jnp.arange(N, dtype=jnp.float32)
        freq = jnp.exp(
            (jnp.arange(D // 2, dtype=jnp.float32) / D)
            * jnp.log(jnp.float32(10000.0)))
        x = pos[:, None] / freq
        pe = jnp.stack((jnp.sin(x), jnp.cos(x)), axis=-1)
        return np.asarray(pe.reshape(N, D), dtype=np.float32)


def _fp16(a):
    return np.ascontiguousarray(np.asarray(a, dtype=np.float32).astype(np.float16))


def kernel(video_features, text_features, Wq, Wk, Wv, Wout):
    global LAST_RESULT
    if "nc" not in _CACHE:
        _CACHE["nc"] = _build()
        _CACHE["pe"] = _pos_encoding()
    nc = _CACHE["nc"]

    vT = _fp16(np.asarray(video_features, dtype=np.float32).T)
    tT = _fp16(np.asarray(text_features, dtype=np.float32).T)
    Wq = np.asarray(Wq, dtype=np.float32)
    Wk = np.asarray(Wk, dtype=np.float32)
    Wv = np.asarray(Wv, dtype=np.float32)
    Wout = np.asarray(Wout, dtype=np.float32)

    in_maps = []
    for h in range(NCORES):
        in_maps.append({
            "vT": vT,
            "tT": tT,
            "wq": _fp16(Wq[h]),
            "wk": _fp16(Wk[h]),
            "wv": _fp16(Wv[h]),
            "wo": _fp16(Wout[h * DH:(h + 1) * DH, :]),
        })
    res = run_bass_kernel_spmd(nc, in_maps, list(range(NCORES)), trace=TRACE)
    LAST_RESULT = res
    acc = res.results[0]["out"].astype(np.float32)
    for h in range(1, NCORES):
        acc = acc + res.results[h]["out"].astype(np.float32)
    return (acc + _CACHE["pe"]).astype(np.float32)



# revision 4
# speedup vs baseline: 1.2409x; 1.2409x over previous
"""Trainium2 Bass kernel for an 8-head cross-attention block.

Math (per reference):
    Q = video @ Wq[h]           [4096, 64]  per head
    K = text  @ Wk[h]           [1024, 64]
    V = text  @ Wv[h]           [1024, 64]
    att = softmax(Q @ K^T)      [4096, 1024]   (no scaling)
    y_h = att @ V               [4096, 64]
    out = concat_h(y_h) @ Wout + pos_enc(4096, 512)

Sharding: 4 head-groups x 2 query-groups over 8 cores. Core c owns heads
(2*(c//2), 2*(c//2)+1) and queries [(c%2)*2048, (c%2+1)*2048). Each core
emits a full-width [2048, 512] partial of the output projection; the host
sums the 4 head-group partials per query half and adds the positional
encoding.

Two heads per core makes every non-attention matmul full-width on the PE:
the head pair occupies PE columns 0-63/64-127 in the Q/K projections
(stationary [Wq_h0|Wq_h1]) and PE rows 0-63/64-127 in the output
projection (K=128 contraction over both heads' y^T at once). PE cost is
free-dim passes only, so the binding per-core work is the E and att@V
stages (fixed by the math) plus exp on the scalar engine.

Softmax: P = exp(E - 12) via ScalarE on [128, 1024] PSUM tiles; the
denominator comes free as a 65th att@V output row (ones column baked into
the V stationary as [V_h0 | ones | V_h1], host supplies the zero slot).
Per-query normalization happens on the [dh, q] y tiles: DVE reciprocal of
the den row, GpSimd partition-broadcast to 64 rows, one fused DVE
multiply+cast into y_sb. Everything runs fp16 operands with fp32 PSUM.
"""

import numpy as np

from concourse import bacc
import concourse.mybir as mybir
from concourse.tile import TileContext
from concourse.bass_utils import run_bass_kernel_spmd

N, M, D, H, DH = 4096, 1024, 512, 8, 64
P = 128
NL = N // 2          # queries per core (2 query groups)
DC = D // P          # 4 contraction chunks of 128
MT = M // P          # 8 key tiles of 128
NJ = NL // 512       # 4 query chunks of 512 per core
F32 = mybir.dt.float32
FP16 = mybir.dt.float16
EXP = mybir.ActivationFunctionType.Exp
EXP_SHIFT = -12.0  # exp(E + shift): keeps fp16 in range; cancels in softmax
NCORES = 8

_CACHE: dict = {}
TRACE = False          # test harness can flip this before calling kernel()
LAST_RESULT = None     # BassKernelResults of the last run (for profiling)


def _body(tc, nc, vT, tT, wq2, wk2, wv3, wo2, out):
    with tc.tile_pool(name="const", bufs=1) as cp:
        vt_sb = cp.tile([P, DC * NL], FP16, tag="vt")
        tt_sb = cp.tile([P, DC * M], FP16, tag="tt")
        wq_sb = cp.tile([P, DC * P], FP16, tag="wq")
        wk_sb = cp.tile([P, DC * P], FP16, tag="wk")
        wv_sb = cp.tile([P, DC * 130], FP16, tag="wv")
        wo_sb = cp.tile([P, D], FP16, tag="wo")
        qt_sb = cp.tile([P, NL], FP16, tag="qt")     # rows: h0 dh | h1 dh
        kt_sb = cp.tile([P, M], FP16, tag="kt")      # rows: h0 dh | h1 dh
        v_sb = cp.tile([P, MT * 130], FP16, tag="v")  # [V_h0|ones|V_h1|ones]
        y_sb = cp.tile([P, NL], FP16, tag="y")       # rows: h0 dh | h1 dh
        bias_sb = cp.tile([P, 1], F32, tag="bias")

        for c in range(DC):
            nc.sync.dma_start(out=wk_sb[:, c * P:(c + 1) * P],
                              in_=wk2[c * P:(c + 1) * P, :])
            nc.sync.dma_start(out=tt_sb[:, c * M:(c + 1) * M],
                              in_=tT[c * P:(c + 1) * P, :])
            nc.sync.dma_start(out=wq_sb[:, c * P:(c + 1) * P],
                              in_=wq2[c * P:(c + 1) * P, :])
            nc.sync.dma_start(out=wv_sb[:, c * 130:(c + 1) * 130],
                              in_=wv3[c * P:(c + 1) * P, :])
        for c in range(DC):
            nc.sync.dma_start(out=vt_sb[:, c * NL:(c + 1) * NL],
                              in_=vT[c * P:(c + 1) * P, :])
        nc.sync.dma_start(out=wo_sb[:, :], in_=wo2[:, :])

        v3 = v_sb.rearrange("p (m e) -> p m e", e=130)  # [128, 8, 130]
        nc.vector.memset(bias_sb[:, :], EXP_SHIFT)

        # ---- projections: K^T [128,1024], Q^T [128,2048], V' [128,8,129] ----
        with tc.tile_pool(name="ps_proj", bufs=3, space="PSUM") as pj:
            for mh in range(M // 512):
                ps = pj.tile([P, 512], F32, tag="ps")
                for c in range(DC):
                    nc.tensor.matmul(
                        ps[:, :],
                        wk_sb[:, c * P:(c + 1) * P],
                        tt_sb[:, c * M + mh * 512: c * M + (mh + 1) * 512],
                        start=(c == 0), stop=(c == DC - 1))
                nc.vector.tensor_copy(
                    out=kt_sb[:, mh * 512:(mh + 1) * 512], in_=ps[:, :])
            for j in range(NJ):
                ps = pj.tile([P, 512], F32, tag="ps")
                for c in range(DC):
                    nc.tensor.matmul(
                        ps[:, :],
                        wq_sb[:, c * P:(c + 1) * P],
                        vt_sb[:, c * NL + j * 512: c * NL + (j + 1) * 512],
                        start=(c == 0), stop=(c == DC - 1))
                nc.vector.tensor_copy(
                    out=qt_sb[:, j * 512:(j + 1) * 512], in_=ps[:, :])
            for mt in range(MT):
                ps = pj.tile([P, 130], F32, tag="psv")
                for c in range(DC):
                    nc.tensor.matmul(
                        ps[:, :],
                        tt_sb[:, c * M + mt * P: c * M + (mt + 1) * P],
                        wv_sb[:, c * 130:(c + 1) * 130],
                        start=(c == 0), stop=(c == DC - 1))
                nc.vector.tensor_copy(out=v3[:, mt, :], in_=ps[:, :])
        nc.vector.memset(v3[:, :, DH], 1.0)
        nc.vector.memset(v3[:, :, 129], 1.0)

        # ---- attention: E^T -> exp -> y^T (+den row) -> normalize ----
        with tc.tile_pool(name="ps_e", bufs=3, space="PSUM") as e_pool, \
             tc.tile_pool(name="ps_y", bufs=2, space="PSUM") as y_pool, \
             tc.tile_pool(name="p_sb", bufs=6) as p_pool, \
             tc.tile_pool(name="nrm", bufs=2) as n_pool:
            for j in range(NJ):
                jsl = slice(j * 512, (j + 1) * 512)
                for h in range(2):
                    hs = slice(h * DH, (h + 1) * DH)
                    p_tiles = []
                    for tp in range(MT // 2):
                        e_ps = e_pool.tile([P, 1024], F32, tag="e")
                        for i in range(2):
                            mt = 2 * tp + i
                            nc.tensor.matmul(
                                e_ps[:, i * 512:(i + 1) * 512],
                                kt_sb[hs, mt * P:(mt + 1) * P],
                                qt_sb[hs, jsl],
                                start=True, stop=True)
                        pt = p_pool.tile([P, 1024], FP16, tag="p")
                        nc.scalar.activation(pt[:, :], e_ps[:, :], EXP,
                                             bias=bias_sb[:, :])
                        p_tiles.append(pt)
                    y_ps = y_pool.tile([DH + 1, 512], F32, tag="y")
                    vsl = slice(0, DH + 1) if h == 0 else slice(DH + 1, 130)
                    for mt in range(MT):
                        nc.tensor.matmul(
                            y_ps[:, :],
                            v3[:, mt, vsl],
                            p_tiles[mt // 2][:, (mt % 2) * 512:(mt % 2 + 1) * 512],
                            start=(mt == 0), stop=(mt == MT - 1))
                    # normalize: den is always row 64 of y_ps
                    rden = n_pool.tile([1, 512], F32, tag="rden")
                    bc = n_pool.tile([DH, 512], F32, tag="bc")
                    nc.vector.reciprocal(rden[:, :], y_ps[DH:DH + 1, :])
                    nc.gpsimd.partition_broadcast(bc[:, :], rden[:, :])
                    nc.vector.tensor_mul(y_sb[hs, jsl], y_ps[0:DH, :], bc[:, :])

        # ---- output projection: out[n, d] = y^T(both heads) @ wo2 ----
        out_r = out.rearrange("(g p) d -> p g d", p=P)  # [128, 16, 512]
        with tc.tile_pool(name="ps_o", bufs=4, space="PSUM") as po_pool, \
             tc.tile_pool(name="o_sb", bufs=2) as o_pool:
            for g in range(NL // P // 4):
                ot = o_pool.tile([P, 4 * D], FP16, tag="o")
                for k in range(4):
                    nt = g * 4 + k
                    ps = po_pool.tile([P, D], F32, tag="po")
                    nc.tensor.matmul(
                        ps[:, :],
                        y_sb[:, nt * P:(nt + 1) * P],
                        wo_sb[:, :],
                        start=True, stop=True)
                    nc.vector.tensor_copy(out=ot[:, k * D:(k + 1) * D],
                                          in_=ps[:, :])
                nc.sync.dma_start(
                    out=out_r[:, g * 4:(g + 1) * 4, :],
                    in_=ot.rearrange("p (g d) -> p g d", d=D))


def _build():
    nc = bacc.Bacc("TRN2", target_bir_lowering=False, debug=False)
    vT = nc.dram_tensor("vT", [D, NL], FP16, kind="ExternalInput")
    tT = nc.dram_tensor("tT", [D, M], FP16, kind="ExternalInput")
    wq2 = nc.dram_tensor("wq2", [D, P], FP16, kind="ExternalInput")
    wk2 = nc.dram_tensor("wk2", [D, P], FP16, kind="ExternalInput")
    wv3 = nc.dram_tensor("wv3", [D, 130], FP16, kind="ExternalInput")
    wo2 = nc.dram_tensor("wo2", [P, D], FP16, kind="ExternalInput")
    out = nc.dram_tensor("out", [NL, D], FP16, kind="ExternalOutput")
    with TileContext(nc) as tc:
        _body(tc, nc, vT[:, :], tT[:, :], wq2[:, :], wk2[:, :], wv3[:, :],
              wo2[:, :], out[:, :])
    nc.compile()
    return nc


def _pos_encoding():
    # Mirror the reference's jnp ops bit-for-bit (numpy's f32 sin/exp differ
    # by enough ULPs to dominate the error budget at pos/freq ~ 4e3).
    import jax
    import jax.numpy as jnp
    with jax.default_device(jax.devices("cpu")[0]):
        pos = jnp.arange(N, dtype=jnp.float32)
        freq = jnp.exp(
            (jnp.arange(D // 2, dtype=jnp.float32) / D)
            * jnp.log(jnp.float32(10000.0)))
        x = pos[:, None] / freq
        pe = jnp.stack((jnp.sin(x), jnp.cos(x)), axis=-1)
        return np.asarray(pe.reshape(N, D), dtype=np.float32)


def _fp16(a):
    return np.ascontiguousarray(np.asarray(a, dtype=np.float32).astype(np.float16))


def kernel(video_features, text_features, Wq, Wk, Wv, Wout):
    global LAST_RESULT
    if "nc" not in _CACHE:
        _CACHE["nc"] = _build()
        _CACHE["pe"] = _pos_encoding()
    nc = _CACHE["nc"]

    vT = np.asarray(video_features, dtype=np.float32).T
    tT = _fp16(np.asarray(text_features, dtype=np.float32).T)
    Wq = np.asarray(Wq, dtype=np.float32)
    Wk = np.asarray(Wk, dtype=np.float32)
    Wv = np.asarray(Wv, dtype=np.float32)
    Wout = np.asarray(Wout, dtype=np.float32)
    zcol = np.zeros((D, 1), dtype=np.float32)

    in_maps = []
    for c in range(NCORES):
        hg, qg = c // 2, c % 2
        h0, h1 = 2 * hg, 2 * hg + 1
        in_maps.append({
            "vT": _fp16(vT[:, qg * NL:(qg + 1) * NL]),
            "tT": tT,
            "wq2": _fp16(np.concatenate([Wq[h0], Wq[h1]], axis=1)),
            "wk2": _fp16(np.concatenate([Wk[h0], Wk[h1]], axis=1)),
            "wv3": _fp16(np.concatenate([Wv[h0], zcol, Wv[h1], zcol], axis=1)),
            "wo2": _fp16(Wout[h0 * DH:(h1 + 1) * DH, :]),
        })
    res = run_bass_kernel_spmd(nc, in_maps, list(range(NCORES)), trace=TRACE)
    LAST_RESULT = res
    acc = np.zeros((N, D), dtype=np.float32)
    for c in range(NCORES):
        hg, qg = c // 2, c % 2
        acc[qg * NL:(qg + 1) * NL] += res.results[c]["out"].astype(np.float32)
    return (acc + _CACHE["pe"]).astype(np.float32)


# revision 6
# speedup vs baseline: 1.3805x; 1.1126x over previous
"""Trainium2 Bass kernel for an 8-head cross-attention block.

Math (per reference):
    Q = video @ Wq[h]           [4096, 64]  per head
    K = text  @ Wk[h]           [1024, 64]
    V = text  @ Wv[h]           [1024, 64]
    att = softmax(Q @ K^T)      [4096, 1024]   (no scaling)
    y_h = att @ V               [4096, 64]
    out = concat_h(y_h) @ Wout + pos_enc(4096, 512)

Sharding: 4 head-groups x 2 query-groups over 8 cores. Core c owns heads
(2*(c//2), 2*(c//2)+1) and queries [(c%2)*2048, (c%2+1)*2048). Each core
emits a full-width [2048, 512] partial of the output projection; the host
sums the 4 head-group partials per query half and adds the positional
encoding.

Two heads per core makes every non-attention matmul full-width on the PE:
the head pair occupies PE columns 0-63/64-127 in the Q/K projections
(stationary [Wq_h0|Wq_h1]) and PE rows 0-63/64-127 in the output
projection (K=128 contraction over both heads' y^T at once). PE cost is
free-dim passes only, so the binding per-core work is the E and att@V
stages (fixed by the math) plus exp on the scalar engine.

Softmax: P = exp(E - 12) via ScalarE on [128, 1024] PSUM tiles; the
denominator comes free as a 65th att@V output row (ones column baked into
the V stationary as [V_h0 | ones | V_h1], host supplies the zero slot).
Per-query normalization happens on the [dh, q] y tiles: DVE reciprocal of
the den row, GpSimd partition-broadcast to 64 rows, one fused DVE
multiply+cast into y_sb. Everything runs fp16 operands with fp32 PSUM.
"""

import numpy as np

from concourse import bacc
import concourse.mybir as mybir
from concourse.tile import TileContext
from concourse.bass_utils import run_bass_kernel_spmd

N, M, D, H, DH = 4096, 1024, 512, 8, 64
P = 128
NL = N // 2          # queries per core (2 query groups)
DC = D // P          # 4 contraction chunks of 128
MT = M // P          # 8 key tiles of 128
NJ = NL // 512       # 4 query chunks of 512 per core
F32 = mybir.dt.float32
FP16 = mybir.dt.float16
EXP = mybir.ActivationFunctionType.Exp
EXP_SHIFT = -12.0  # exp(E + shift): keeps fp16 in range; cancels in softmax
NCORES = 8

_CACHE: dict = {}
TRACE = False          # test harness can flip this before calling kernel()
LAST_RESULT = None     # BassKernelResults of the last run (for profiling)


def _body(tc, nc, vT, tT, wq2, wk2, wv3, wo2, out):
    with tc.tile_pool(name="const", bufs=1) as cp:
        vt_sb = cp.tile([P, DC * NL], FP16, tag="vt")
        tt_sb = cp.tile([P, DC * M], FP16, tag="tt")
        wq_sb = cp.tile([P, DC * P], FP16, tag="wq")
        wk_sb = cp.tile([P, DC * P], FP16, tag="wk")
        wv_sb = cp.tile([P, DC * 130], FP16, tag="wv")
        wo_sb = cp.tile([P, D], FP16, tag="wo")
        qt_sb = cp.tile([P, NL], FP16, tag="qt")     # rows: h0 dh | h1 dh
        kt_sb = cp.tile([P, M], FP16, tag="kt")      # rows: h0 dh | h1 dh
        v_sb = cp.tile([P, MT * 130], FP16, tag="v")  # [V_h0|ones|V_h1|ones]
        y_sb = cp.tile([P, NL], FP16, tag="y")       # rows: h0 dh | h1 dh
        bias_sb = cp.tile([P, 1], F32, tag="bias")

        for c in range(DC):
            nc.sync.dma_start(out=wk_sb[:, c * P:(c + 1) * P],
                              in_=wk2[c * P:(c + 1) * P, :])
            nc.sync.dma_start(out=tt_sb[:, c * M:(c + 1) * M],
                              in_=tT[c * P:(c + 1) * P, :])
            nc.sync.dma_start(out=wq_sb[:, c * P:(c + 1) * P],
                              in_=wq2[c * P:(c + 1) * P, :])
            nc.sync.dma_start(out=wv_sb[:, c * 130:(c + 1) * 130],
                              in_=wv3[c * P:(c + 1) * P, :])
        for c in range(DC):
            nc.sync.dma_start(out=vt_sb[:, c * NL:(c + 1) * NL],
                              in_=vT[c * P:(c + 1) * P, :])
        nc.sync.dma_start(out=wo_sb[:, :], in_=wo2[:, :])

        v3 = v_sb.rearrange("p (m e) -> p m e", e=130)  # [128, 8, 130]
        nc.vector.memset(bias_sb[:, :], EXP_SHIFT)

        # ---- projections: K^T [128,1024], Q^T [128,2048], V' [128,8,129] ----
        with tc.tile_pool(name="ps_proj", bufs=3, space="PSUM") as pj:
            for mh in range(M // 512):
                ps = pj.tile([P, 512], F32, tag="ps")
                for c in range(DC):
                    nc.tensor.matmul(
                        ps[:, :],
                        wk_sb[:, c * P:(c + 1) * P],
                        tt_sb[:, c * M + mh * 512: c * M + (mh + 1) * 512],
                        start=(c == 0), stop=(c == DC - 1))
                nc.vector.tensor_copy(
                    out=kt_sb[:, mh * 512:(mh + 1) * 512], in_=ps[:, :])
            for j in range(NJ):
                ps = pj.tile([P, 512], F32, tag="ps")
                for c in range(DC):
                    nc.tensor.matmul(
                        ps[:, :],
                        wq_sb[:, c * P:(c + 1) * P],
                        vt_sb[:, c * NL + j * 512: c * NL + (j + 1) * 512],
                        start=(c == 0), stop=(c == DC - 1))
                nc.vector.tensor_copy(
                    out=qt_sb[:, j * 512:(j + 1) * 512], in_=ps[:, :])
            for mt in range(MT):
                ps = pj.tile([P, 130], F32, tag="psv")
                for c in range(DC):
                    nc.tensor.matmul(
                        ps[:, :],
                        tt_sb[:, c * M + mt * P: c * M + (mt + 1) * P],
                        wv_sb[:, c * 130:(c + 1) * 130],
                        start=(c == 0), stop=(c == DC - 1))
                nc.vector.tensor_copy(out=v3[:, mt, :], in_=ps[:, :])
        nc.vector.memset(v3[:, :, DH], 1.0)
        nc.vector.memset(v3[:, :, 129], 1.0)

        # ---- attention: E^T -> exp -> y^T (+den row) -> normalize ----
        out_r = out.rearrange("(g p) d -> p g d", p=P)  # [128, 16, 512]

        def emit_out(j, po_pool, o_pool):
            for nt in range(j * 4, (j + 1) * 4):
                ps = po_pool.tile([P, D], F32, tag="po")
                nc.tensor.matmul(
                    ps[:, :],
                    y_sb[:, nt * P:(nt + 1) * P],
                    wo_sb[:, :],
                    start=True, stop=True)
                ot = o_pool.tile([P, D], FP16, tag="o")
                nc.vector.tensor_copy(out=ot[:, :], in_=ps[:, :])
                nc.sync.dma_start(out=out_r[:, nt, :], in_=ot[:, :])

        with tc.tile_pool(name="ps_e", bufs=2, space="PSUM") as e_pool, \
             tc.tile_pool(name="ps_y", bufs=2, space="PSUM") as y_pool, \
             tc.tile_pool(name="ps_o", bufs=2, space="PSUM") as po_pool, \
             tc.tile_pool(name="p_sb", bufs=6) as p_pool, \
             tc.tile_pool(name="o_sb", bufs=4) as o_pool, \
             tc.tile_pool(name="nrm", bufs=2) as n_pool:
            for j in range(NJ):
                jsl = slice(j * 512, (j + 1) * 512)
                for h in range(2):
                    if j > 0 and h == 1:
                        emit_out(j - 1, po_pool, o_pool)
                    hs = slice(h * DH, (h + 1) * DH)
                    p_tiles = []
                    for tp in range(MT // 2):
                        e_ps = e_pool.tile([P, 1024], F32, tag="e")
                        for i in range(2):
                            mt = 2 * tp + i
                            nc.tensor.matmul(
                                e_ps[:, i * 512:(i + 1) * 512],
                                kt_sb[hs, mt * P:(mt + 1) * P],
                                qt_sb[hs, jsl],
                                start=True, stop=True)
                        pt = p_pool.tile([P, 1024], FP16, tag="p")
                        nc.scalar.activation(pt[:, :], e_ps[:, :], EXP,
                                             bias=bias_sb[:, :])
                        p_tiles.append(pt)
                    y_ps = y_pool.tile([DH + 1, 512], F32, tag="y")
                    vsl = slice(0, DH + 1) if h == 0 else slice(DH + 1, 130)
                    for mt in range(MT):
                        nc.tensor.matmul(
                            y_ps[:, :],
                            v3[:, mt, vsl],
                            p_tiles[mt // 2][:, (mt % 2) * 512:(mt % 2 + 1) * 512],
                            start=(mt == 0), stop=(mt == MT - 1))
                    # normalize: den is always row 64 of y_ps
                    den = n_pool.tile([1, 512], F32, tag="den")
                    rden = n_pool.tile([1, 512], F32, tag="rden")
                    bc = n_pool.tile([DH, 512], F32, tag="bc")
                    nc.vector.tensor_copy(out=den[:, :], in_=y_ps[DH:DH + 1, :])
                    nc.vector.reciprocal_approx_fast(rden[:, :], den[:, :])
                    nc.gpsimd.partition_broadcast(bc[:, :], rden[:, :])
                    nc.vector.tensor_mul(y_sb[hs, jsl], y_ps[0:DH, :], bc[:, :])
            emit_out(NJ - 1, po_pool, o_pool)


def _build():
    nc = bacc.Bacc("TRN2", target_bir_lowering=False, debug=False)
    vT = nc.dram_tensor("vT", [D, NL], FP16, kind="ExternalInput")
    tT = nc.dram_tensor("tT", [D, M], FP16, kind="ExternalInput")
    wq2 = nc.dram_tensor("wq2", [D, P], FP16, kind="ExternalInput")
    wk2 = nc.dram_tensor("wk2", [D, P], FP16, kind="ExternalInput")
    wv3 = nc.dram_tensor("wv3", [D, 130], FP16, kind="ExternalInput")
    wo2 = nc.dram_tensor("wo2", [P, D], FP16, kind="ExternalInput")
    out = nc.dram_tensor("out", [NL, D], FP16, kind="ExternalOutput")
    with TileContext(nc) as tc:
        _body(tc, nc, vT[:, :], tT[:, :], wq2[:, :], wk2[:, :], wv3[:, :],
              wo2[:, :], out[:, :])
    nc.compile()
    return nc


def _pos_encoding():
    # Mirror the reference's jnp ops bit-for-bit (numpy's f32 sin/exp differ
    # by enough ULPs to dominate the error budget at pos/freq ~ 4e3).
    import jax
    import jax.numpy as jnp
    with jax.default_device(jax.devices("cpu")[0]):
        pos = jnp.arange(N, dtype=jnp.float32)
        freq = jnp.exp(
            (jnp.arange(D // 2, dtype=jnp.float32) / D)
            * jnp.log(jnp.float32(10000.0)))
        x = pos[:, None] / freq
        pe = jnp.stack((jnp.sin(x), jnp.cos(x)), axis=-1)
        return np.asarray(pe.reshape(N, D), dtype=np.float32)


def _fp16(a):
    return np.ascontiguousarray(np.asarray(a, dtype=np.float32).astype(np.float16))


def kernel(video_features, text_features, Wq, Wk, Wv, Wout):
    global LAST_RESULT
    if "nc" not in _CACHE:
        _CACHE["nc"] = _build()
        _CACHE["pe"] = _pos_encoding()
    nc = _CACHE["nc"]

    vT = np.asarray(video_features, dtype=np.float32).T
    tT = _fp16(np.asarray(text_features, dtype=np.float32).T)
    Wq = np.asarray(Wq, dtype=np.float32)
    Wk = np.asarray(Wk, dtype=np.float32)
    Wv = np.asarray(Wv, dtype=np.float32)
    Wout = np.asarray(Wout, dtype=np.float32)
    zcol = np.zeros((D, 1), dtype=np.float32)

    in_maps = []
    for c in range(NCORES):
        hg, qg = c // 2, c % 2
        h0, h1 = 2 * hg, 2 * hg + 1
        in_maps.append({
            "vT": _fp16(vT[:, qg * NL:(qg + 1) * NL]),
            "tT": tT,
            "wq2": _fp16(np.concatenate([Wq[h0], Wq[h1]], axis=1)),
            "wk2": _fp16(np.concatenate([Wk[h0], Wk[h1]], axis=1)),
            "wv3": _fp16(np.concatenate([Wv[h0], zcol, Wv[h1], zcol], axis=1)),
            "wo2": _fp16(Wout[h0 * DH:(h1 + 1) * DH, :]),
        })
    res = run_bass_kernel_spmd(nc, in_maps, list(range(NCORES)), trace=TRACE)
    LAST_RESULT = res
    acc = np.zeros((N, D), dtype=np.float32)
    for c in range(NCORES):
        hg, qg = c // 2, c % 2
        acc[qg * NL:(qg + 1) * NL] += res.results[c]["out"].astype(np.float32)
    return (acc + _CACHE["pe"]).astype(np.float32)
